# revision 26
# baseline (speedup 1.0000x reference)
"""Trainium2 Bass kernel for the GNN message-passing model.

Strategy: data-parallel over batch (B=16 -> 2 batches per core, 8 cores, no
cross-core communication). The adjacency matrix is pre-scaled by N (its
entries are ~1/N, below the fp8 subnormal range), cast to fp8-e4m3 on the
host and kept FULLY RESIDENT in SBUF (16 MB -> 128 KB/partition). The
adjacency matmul runs in fp8 DoubleRow perf mode (K=256 per matmul) with
the cut features as the stationary operand, scaled per layer into fp8
range; scales are divided back out in the PSUM drain. Cut features are
produced directly node-major (stationary = x node-tile, moving = gw cut
columns), eliminating all PE transposes. Half of each layer's adjacency
matmul (i-block group 0) is interleaved under the producer loop that
builds the stationary features, keeping the PE dense (HAM-warm) and
hiding ~16 us per layer.

Host-side folds: pw3/emb folded into gw0; action-MLP output folded into
the embedding table via the one-hot rows summing to 1; the nerf frequency
scaling/replication of mesh is a host-side linear layout transform.
"""

import numpy as np
import ml_dtypes

import concourse.bass as bass
import concourse.mybir as mybir
import concourse.tile as tile
from concourse.masks import make_identity
from concourse.bass_utils import run_bass_kernel_spmd

F32 = mybir.dt.float32
BF16 = mybir.dt.bfloat16
FP8 = mybir.dt.float8e4
AF = mybir.ActivationFunctionType
ALU = mybir.AluOpType
PM = mybir.MatmulPerfMode
BF = ml_dtypes.bfloat16
F8 = ml_dtypes.float8_e4m3

B, N, BC = 16, 4096, 2          # batches, nodes, batches per core
NCORES = 8
NB = 8                          # i-blocks / chunks of 512 columns
NQ = 32                         # node tiles of 128
NT = 16                         # double-j-tiles of 256 rows
MAGIC = float(1.5 * 2 ** 23)    # fp32 round-to-nearest magic constant
TWO_PI = float(2.0 * np.pi)
SA = 4096.0                     # adj pre-scale (host)
SF = [256.0, 512.0, 1024.0, 1024.0]   # f_cut fp8 scales per layer
CD = [42, 42, 42, 50]           # cut widths

# const blob layouts: name -> (rows, col offset, cols)
O16 = {"pw1p": (67, 0, 25), "pw2": (25, 25, 50), "w3f": (50, 75, 128),
       "t4": (4, 203, 128), "gw1": (128, 331, 128), "gw2": (128, 459, 128),
       "gw3": (128, 587, 50), "ones4": (2, 637, 8)}
W16 = 645
O32 = {"aw1a": (50, 0, 128), "aw1b": (50, 128, 72), "aw2a": (128, 200, 100),
       "aw2b": (72, 300, 100), "aw3": (100, 400, 100),
       "gw0L": (100, 500, 128), "biasd": (128, 628, 12)}
W32 = 640

run_kwargs = {}                 # test.py may inject trace kwargs here


def split_excess_waits(nc, max_waits=1):
    """Walrus codegen on this image rejects >1 sem wait per instruction;
    move excess waits onto preceding same-engine no-ops."""
    n_split = 0
    for fn in nc.m.functions:
        for blk in fn.blocks:
            insts = list(blk.instructions)
            out = []
            changed = False
            for inst in insts:
                si = getattr(inst, "sync_info", None)
                if si is not None and len(si.on_wait) > max_waits:
                    waits = list(si.on_wait)
                    chunks = [waits[i:i + max_waits]
                              for i in range(0, len(waits), max_waits)]
                    for ci, ch in enumerate(chunks[:-1]):
                        nop = mybir.InstNoOp(
                            name=f"{inst.name}-wsplit-{ci}", ins=[], outs=[])
                        nop.engine = inst.engine
                        nop.sync_info = mybir.SyncInfo(on_wait=ch, on_update=[])
                        out.append(nop)
                        n_split += 1
                    inst.sync_info = mybir.SyncInfo(
                        on_wait=chunks[-1], on_update=list(si.on_update))
                    changed = True
                out.append(inst)
            if changed:
                blk.instructions = out
    return n_split


def build_bass(split=True):
    nc = bass.Bass()

    def _param(name, shape, dt):
        return nc.declare_dram_parameter(name, list(shape), dt, isOutput=False)

    adj8d = _param("adj8d", [128, NQ * N], FP8)
    meshTd = _param("meshTd", [BC, 3, N], F32)
    mshftd = _param("mshftd", [128, N], F32)
    onehotd = _param("onehotd", [4, BC * N], BF16)
    maskTd = _param("maskTd", [50, BC], F32)
    wb16d = _param("wb16d", [128, W16], BF16)
    wb32d = _param("wb32d", [128, W32], F32)
    outd = nc.declare_dram_parameter("outd", [114, 1], F32, isOutput=True)

    with tile.TileContext(nc) as tc:
        _emit(nc, tc, locals())
    if split:
        split_excess_waits(nc)
    return nc


def _emit(nc, tc, d):
    import contextlib
    ctx = contextlib.ExitStack()
    meshTd, onehotd, outd, mshftd = (d["meshTd"], d["onehotd"], d["outd"],
                                     d["mshftd"])

    adjp = ctx.enter_context(tc.tile_pool(name="adjp", bufs=1))
    cpool = ctx.enter_context(tc.tile_pool(name="consts", bufs=1))
    actp = ctx.enter_context(tc.tile_pool(name="acts", bufs=1))
    fcp = ctx.enter_context(tc.tile_pool(name="fcp", bufs=2))
    wkp = ctx.enter_context(tc.tile_pool(name="wkp", bufs=2))
    smallp = ctx.enter_context(tc.tile_pool(name="small", bufs=2))

    psf = ctx.enter_context(tc.tile_pool(name="psf", bufs=2, space="PSUM"))
    psc = ctx.enter_context(tc.tile_pool(name="psc", bufs=2, space="PSUM"))
    psl = ctx.enter_context(tc.tile_pool(name="psl", bufs=4, space="PSUM"))

    # ------- small latency-critical DMAs first (sync queue head) -------
    wb16 = cpool.tile([128, W16], BF16, tag="wb16")
    nc.sync.dma_start(out=wb16[:], in_=d["wb16d"][:])
    wb32 = cpool.tile([128, W32], F32, tag="wb32")
    nc.sync.dma_start(out=wb32[:], in_=d["wb32d"][:])
    maskT = cpool.tile([50, BC], F32, tag="maskT")
    nc.sync.dma_start(out=maskT[:], in_=d["maskTd"][:])
    onehot = cpool.tile([4, BC * N], BF16, tag="onehot")
    nc.sync.dma_start(out=onehot[:], in_=onehotd[:])

    # ------- resident fp8 adjacency streams behind them on sync -------
    adjsb = adjp.tile([128, NQ * N], FP8, tag="adj")
    a3 = adjsb[:].rearrange("p (q c) -> p q c", c=N)
    a3d = d["adj8d"][:].rearrange("p (q c) -> p q c", c=N)
    for g in range(8):
        nc.sync.dma_start(out=a3[:, 4 * g:4 * g + 4, :],
                          in_=a3d[:, 4 * g:4 * g + 4, :])

    def c16(key):
        r, o, w = O16[key]
        return wb16[0:r, o:o + w]

    def c32(key):
        r, o, w = O32[key]
        return wb32[0:r, o:o + w]

    ident = cpool.tile([128, 128], BF16, tag="ident")
    make_identity(nc, ident[:])
    pw1, pw2, w3f, t4 = c16("pw1p"), c16("pw2"), c16("w3f"), c16("t4")
    ones4 = c16("ones4")
    gws = [None, c16("gw1"), c16("gw2"), c16("gw3")]
    biases = c32("biasd")

    def bcol(col, p0, p1):
        return biases[p0:p1, col:col + 1]

    # ---------------- activation tiles ----------------
    xt = actp.tile([128, BC * N], BF16, tag="xt")      # [feat, b*N+n]
    mx = actp.tile([114, NB], F32, tag="mx")
    nc.gpsimd.memset(mx[:], 0.0)
    outsb = actp.tile([114, 1], F32, tag="outsb")
    cvec = actp.tile([128, BC], BF16, tag="cvec")
    cvT = actp.tile([2, 128], BF16, tag="cvT")
    t4b = [actp.tile([4, 128], BF16, tag=f"t4b{b}", name=f"t4b{b}")
           for b in range(BC)]

    # ---------------- action MLP (tiny, fp32) ----------------
    pa = psf.tile([128, 2], F32, tag="f")
    nc.tensor.matmul(pa[:], lhsT=c32("aw1a"), rhs=maskT[:], start=True,
                     stop=True)
    a1a = smallp.tile([128, 2], F32, tag="a1a")
    nc.scalar.activation(a1a[:], pa[:], AF.Relu, bias=bcol(0, 0, 128))
    pb = psf.tile([72, 2], F32, tag="f")
    nc.tensor.matmul(pb[:], lhsT=c32("aw1b"), rhs=maskT[:], start=True,
                     stop=True)
    a1b = smallp.tile([72, 2], F32, tag="a1b")
    nc.scalar.activation(a1b[:], pb[:], AF.Relu, bias=bcol(1, 0, 72))
    pc_ = psf.tile([100, 2], F32, tag="f")
    nc.tensor.matmul(pc_[:], lhsT=c32("aw2a"), rhs=a1a[:], start=True,
                     stop=False)
    nc.tensor.matmul(pc_[:], lhsT=c32("aw2b"), rhs=a1b[:], start=False,
                     stop=True)
    a2 = smallp.tile([100, 2], F32, tag="a2")
    nc.scalar.activation(a2[:], pc_[:], AF.Relu, bias=bcol(2, 0, 100))
    pd = psf.tile([100, 2], F32, tag="f")
    nc.tensor.matmul(pd[:], lhsT=c32("aw3"), rhs=a2[:], start=True, stop=True)
    a3t = smallp.tile([100, 2], F32, tag="a3t")
    nc.scalar.activation(a3t[:], pd[:], AF.Identity, bias=bcol(3, 0, 100))
    pe_ = psf.tile([128, 2], F32, tag="f")
    nc.tensor.matmul(pe_[:], lhsT=c32("gw0L"), rhs=a3t[:], start=True,
                     stop=True)
    nc.scalar.activation(cvec[:], pe_[:], AF.Identity, bias=bcol(4, 0, 128))

    # cvec -> cvT (transpose), then t4b[b] = t4 + cvT[b] (one-hot rows sum
    # to 1, so adding cvec to every embedding row injects the per-batch
    # action constant into the layer-0 matmul)
    pt = psc.tile([2, 128], BF16, tag="c")
    nc.tensor.transpose(pt[:], cvec[:], ident[:])
    nc.vector.tensor_copy(cvT[:], pt[:])
    for b in range(BC):
        p4 = psc.tile([4, 128], F32, tag="c")
        nc.tensor.matmul(p4[:], lhsT=ones4[:, 4 * b:4 * b + 4], rhs=cvT[:],
                         start=True, stop=False)
        nc.tensor.matmul(p4[:], lhsT=ident[0:4, 0:4], rhs=t4, start=False,
                         stop=True)
        nc.vector.tensor_copy(t4b[b][:], p4[:])

    # fcst: [128 nodes-in-tile, NT blocks of (2 half, 128 m)] fp8 stationary
    fcst, fvs = [None] * 4, [None] * 4

    def alloc_fcst(li):
        fcst[li] = fcp.tile([128, NT * 256], FP8, tag="fc", name=f"fcst{li}")
        fvs[li] = fcst[li][:].rearrange("p (t h m) -> p t h m", h=2, m=128)
        # zero the gap columns between the two batches' packed features
        nc.vector.memset(fvs[li][:, :, :, 42:64], 0.0)

    def phase_c_mms(li, ts, pls, g):
        mm = 64 + CD[li]
        for t in ts:
            lhsT = fvs[li][:, t, :, 0:mm]
            for k in range(4):
                ib = 4 * g + k
                nc.tensor.matmul(
                    pls[k][:], lhsT=lhsT,
                    rhs=a3[:, 2 * t:2 * t + 2, ib * 512:(ib + 1) * 512],
                    start=(t == 0), stop=(t == NT - 1),
                    perf_mode=PM.DoubleRow)

    def drains(li, pls, g):
        s_li = 1.0 / (SA * SF[li])
        for k in range(4):
            ib = 4 * g + k
            if li < 3:
                for b in range(BC):
                    ics = slice(b * N + ib * 512, b * N + (ib + 1) * 512)
                    nc.scalar.activation(
                        xt[0:42, ics], pls[k][64 * b:64 * b + 42, :],
                        AF.Relu, bias=bcol(7 + li, 64 * b, 64 * b + 42),
                        scale=s_li)
            else:
                nc.vector.tensor_reduce(mx[0:114, ib:ib + 1], pls[k][:],
                                        mybir.AxisListType.X, ALU.max)

    def new_pls(li, g):
        mm = 64 + CD[li]
        return [psl.tile([mm, 512], F32, tag="L", name=f"pl{li}_{g}_{k}")
                for k in range(4)]

    # ---------------- front-end + layer-0 phase A/cut (fused),
    # with layer-0 phase C group 0 interleaved ----------------
    alloc_fcst(0)
    pls0 = new_pls(0, 0)
    for ch in range(NB):
        cs = slice(ch * 512, (ch + 1) * 512)
        # t2[64b+k] = mesh[b,k%3]*freq[k//3]/(2pi) (+0.25 on cos rows),
        # prebuilt host-side; vector-engine DMA so the adj stream on the
        # sync queue cannot delay it
        t2 = wkp.tile([128, 512], F32, tag="t2")
        nc.gpsimd.dma_start(out=t2[:], in_=mshftd[:, cs])
        # range reduce: rB = round(t2) - t2 = -frac; sin(-2pi*rB) = sin(2pi*t2)
        rA = wkp.tile([128, 512], F32, tag="rA")
        nc.vector.tensor_scalar_add(rA[:], t2[:], MAGIC)
        rB = wkp.tile([128, 512], F32, tag="rB")
        nc.vector.scalar_tensor_tensor(rB[:], rA[:], MAGIC, t2[:],
                                       op0=ALU.subtract, op1=ALU.subtract)
        peins, h1s, h2s, ph1s, ph2s, pcus, pfs = [], [], [], [], [], [], []
        for b in range(BC):
            pein = wkp.tile([67, 512], BF16, tag=f"pein{b}", name=f"pein{b}")
            nc.scalar.activation(pein[0:64, :], rB[64 * b:64 * b + 64, :],
                                 AF.Sin, scale=-TWO_PI)
            nc.gpsimd.dma_start(out=pein[64:67, :], in_=meshTd[b, :, cs])
            peins.append(pein)
        for b in range(BC):
            ph1 = psf.tile([25, 512], F32, tag="f")
            nc.tensor.matmul(ph1[:], lhsT=pw1, rhs=peins[b][:], start=True,
                             stop=True)
            ph1s.append(ph1)
        for b in range(BC):
            h1 = wkp.tile([25, 512], BF16, tag=f"h1{b}", name=f"h1{b}")
            nc.vector.tensor_scalar(h1[:], ph1s[b][:], bcol(5, 0, 25), 0.0,
                                    op0=ALU.add, op1=ALU.max)
            h1s.append(h1)
        for b in range(BC):
            ph2 = psf.tile([50, 512], F32, tag="f")
            nc.tensor.matmul(ph2[:], lhsT=pw2, rhs=h1s[b][:], start=True,
                             stop=True)
            ph2s.append(ph2)
        for b in range(BC):
            h2 = wkp.tile([50, 512], BF16, tag=f"h2{b}", name=f"h2{b}")
            nc.scalar.activation(h2[:], ph2s[b][:], AF.Relu,
                                 bias=bcol(6, 0, 50))
            h2s.append(h2)
        # layer-0 cut, node-major: fcT[n,c] = h2.T @ w3f_cut + oh.T @ t4b_cut
        for b in range(BC):
            pcu = psc.tile([128, 4 * 42], F32, tag="c")
            for j in range(4):
                q = 4 * ch + j
                js = slice(42 * j, 42 * j + 42)
                nc.tensor.matmul(pcu[:, js],
                                 lhsT=h2s[b][:, 128 * j:128 * j + 128],
                                 rhs=w3f[:, 0:42], start=True, stop=False)
                nc.tensor.matmul(
                    pcu[:, js],
                    lhsT=onehot[:, b * N + q * 128:b * N + (q + 1) * 128],
                    rhs=t4b[b][:, 0:42], start=False, stop=True)
            pcus.append(pcu)
        for b in range(BC):
            nc.vector.tensor_scalar_mul(
                fvs[0][:, 2 * ch:2 * ch + 2, :, 64 * b:64 * b + 42],
                pcus[b][:].rearrange("p (t h c) -> p t h c", h=2, c=42),
                SF[0])
        # layer-0 right features
        for b in range(BC):
            xs = slice(b * N + ch * 512, b * N + (ch + 1) * 512)
            pf = psf.tile([128, 512], F32, tag="f")
            nc.tensor.matmul(pf[:], lhsT=w3f, rhs=h2s[b][:], start=True,
                             stop=False)
            nc.tensor.matmul(pf[:], lhsT=t4b[b][:], rhs=onehot[:, xs],
                             start=False, stop=True)
            pfs.append(pf)
        for b in range(BC):
            xs = slice(b * N + ch * 512, b * N + (ch + 1) * 512)
            nc.vector.tensor_scalar_max(xt[32:64, xs], pfs[b][32:64, :], 0.0)
            nc.scalar.activation(xt[64:128, xs], pfs[b][64:128, :], AF.Relu)
        phase_c_mms(0, (2 * ch, 2 * ch + 1), pls0, g=0)

    # ---------------- GCN layers ----------------
    for li in range(4):
        if li > 0:
            # phase A (cut + right) with phase C group 0 interleaved
            cd = CD[li]
            alloc_fcst(li)
            pls0 = new_pls(li, 0)
            for ch in range(NB):
                pcus, pfs = [], []
                for b in range(BC):
                    pcu = psc.tile([128, 4 * cd], F32, tag="c")
                    for j in range(4):
                        q = 4 * ch + j
                        nc.tensor.matmul(
                            pcu[:, cd * j:cd * j + cd],
                            lhsT=xt[:, b * N + q * 128:b * N + (q + 1) * 128],
                            rhs=gws[li][:, 0:cd], start=True, stop=True)
                    pcus.append(pcu)
                if li < 3:
                    for b in range(BC):
                        xs = slice(b * N + ch * 512, b * N + (ch + 1) * 512)
                        pf = psf.tile([128, 512], F32, tag="f")
                        nc.tensor.matmul(pf[:], lhsT=gws[li], rhs=xt[:, xs],
                                         start=True, stop=True)
                        pfs.append(pf)
                for b in range(BC):
                    nc.vector.tensor_scalar_mul(
                        fvs[li][:, 2 * ch:2 * ch + 2, :, 64 * b:64 * b + cd],
                        pcus[b][:].rearrange("p (t h c) -> p t h c",
                                             h=2, c=cd), SF[li])
                if li < 3:
                    for b in range(BC):
                        xs = slice(b * N + ch * 512, b * N + (ch + 1) * 512)
                        nc.vector.tensor_scalar_max(xt[32:64, xs],
                                                    pfs[b][32:64, :], 0.0)
                        nc.scalar.activation(xt[64:128, xs],
                                             pfs[b][64:128, :], AF.Relu)
                phase_c_mms(li, (2 * ch, 2 * ch + 1), pls0, g=0)
        # phase C group 0 drains, then group 1 dense
        drains(li, pls0, g=0)
        pls1 = new_pls(li, 1)
        phase_c_mms(li, range(NT), pls1, g=1)
        drains(li, pls1, g=1)

    # ---------------- final max + scale + bias + output ----------------
    mxr = smallp.tile([114, 1], F32, tag="mxr")
    nc.vector.tensor_reduce(mxr[:], mx[:], mybir.AxisListType.X, ALU.max)
    nc.scalar.activation(outsb[:], mxr[:], AF.Identity,
                         bias=bcol(10, 0, 114), scale=1.0 / (SA * SF[3]))
    nc.sync.dma_start(out=outd[:], in_=outsb[:])
    ctx.close()


# ---------------------------------------------------------------------------
# host side
# ---------------------------------------------------------------------------

def _prep_shared(inp):
    """Host preprocessing shared across cores (weights + adj)."""
    f32 = np.float32
    adjT = np.ascontiguousarray(inp["adj"].astype(f32).T)
    adj8 = np.clip(adjT * SA, 0, 240).astype(F8)            # [j, i]
    adj8 = np.ascontiguousarray(
        adj8.reshape(NQ, 128, N).transpose(1, 0, 2)).reshape(128, NQ * N)

    gw0 = inp["gw0"].astype(f32)
    w3fold = inp["pw3"].astype(f32) @ gw0[100:200]
    t4 = inp["emb"].astype(f32) @ gw0[200:300]
    pb3f = inp["pb3"].astype(f32) @ gw0[100:200]

    # pe_in row permutation: ours = [sin(f,c) x30 | cos(f,c) x30 | mesh x3]
    pw1f = inp["pw1"].astype(f32)
    pw1p = np.zeros((67, 25), f32)
    for k in range(30):
        f, c = divmod(k, 3)
        pw1p[k] = pw1f[f * 6 + c]          # sin rows
        pw1p[32 + k] = pw1f[f * 6 + 3 + c]  # cos rows
    pw1p[64:67] = pw1f[60:63]

    biasd = np.zeros((128, 12), f32)
    biasd[0:128, 0] = inp["ab1"][:128]
    biasd[0:72, 1] = inp["ab1"][128:200]
    biasd[0:100, 2] = inp["ab2"]
    biasd[0:100, 3] = inp["ab3"]
    biasd[0:128, 4] = pb3f
    biasd[0:25, 5] = inp["pb1"].astype(f32)
    biasd[0:50, 6] = inp["pb2"].astype(f32)
    for li in range(3):
        biasd[0:42, 7 + li] = inp[f"gb{li}"].astype(f32)[:42]
        biasd[64:106, 7 + li] = inp[f"gb{li}"].astype(f32)[:42]
    biasd[0:50, 10] = inp["gb3"].astype(f32)
    biasd[64:114, 10] = inp["gb3"].astype(f32)

    ones4 = np.zeros((2, 8), f32)
    ones4[0, 0:4] = 1.0
    ones4[1, 4:8] = 1.0

    vals16 = {"pw1p": pw1p, "pw2": inp["pw2"].astype(f32), "w3f": w3fold,
              "t4": t4, "gw1": inp["gw1"].astype(f32),
              "gw2": inp["gw2"].astype(f32), "gw3": inp["gw3"].astype(f32),
              "ones4": ones4}
    wb16 = np.zeros((128, W16), f32)
    for k, (r, o, w) in O16.items():
        wb16[0:r, o:o + w] = vals16[k]
    vals32 = {"aw1a": inp["aw1"].astype(f32)[:, :128],
              "aw1b": inp["aw1"].astype(f32)[:, 128:200],
              "aw2a": inp["aw2"].astype(f32)[:128],
              "aw2b": inp["aw2"].astype(f32)[128:200],
              "aw3": inp["aw3"].astype(f32),
              "gw0L": gw0[:100], "biasd": biasd}
    wb32 = np.zeros((128, W32), f32)
    for k, (r, o, w) in O32.items():
        wb32[0:r, o:o + w] = vals32[k]

    return {
        "adj8d": adj8,
        "wb16d": wb16.astype(BF),
        "wb32d": wb32,
    }


def _prep_core(inp, shared, core):
    bs = slice(core * BC, (core + 1) * BC)
    f32 = np.float32
    mesh = inp["mesh"].astype(f32)[bs]                       # [2, N, 3]
    meshT = np.ascontiguousarray(mesh.transpose(0, 2, 1))    # [2, 3, N]
    mi = inp["mask_idx"][bs]                                 # [2, N] int32
    onehot = (mi[:, None, :] == np.arange(4, dtype=mi.dtype)[None, :, None])
    onehot = np.ascontiguousarray(
        onehot.transpose(1, 0, 2).reshape(4, BC * N)).astype(BF)
    maskT = np.ascontiguousarray(inp["mask"].astype(f32)[bs].T)  # [50, 2]
    # t2 rows, prebuilt: rows 64b+k = freq2[k]*mesh[b, k%3]; +0.25 on cos rows
    freqs = np.asarray([np.pi] + [2.0 * np.pi * i for i in range(1, 10)], f32)
    freq2 = np.repeat(freqs, 3) / (2.0 * np.pi)              # [30]
    mshft = np.zeros((128, N), f32)
    for b in range(BC):
        rep = freq2[:, None] * meshT[b][np.tile(np.arange(3), 10)]  # [30, N]
        mshft[64 * b:64 * b + 30] = rep
        mshft[64 * b + 32:64 * b + 62] = rep + 0.25
    m = dict(shared)
    m["meshTd"] = meshT
    m["mshftd"] = mshft
    m["onehotd"] = onehot
    m["maskTd"] = maskT
    return m


_CACHED = {}


def kernel(**inputs) -> np.ndarray:
    if "nc" not in _CACHED:
        _CACHED["nc"] = build_bass()
    nc = _CACHED["nc"]
    shared = _prep_shared(inputs)
    in_maps = [_prep_core(inputs, shared, c) for c in range(NCORES)]
    res = run_bass_kernel_spmd(nc, in_maps, list(range(NCORES)), **run_kwargs)
    out = np.empty((B, 50), np.float32)
    for c in range(NCORES):
        o = res.results[c]["outd"][:, 0]
        out[2 * c] = o[0:50]
        out[2 * c + 1] = o[64:114]
    _CACHED["last_results"] = res
    return out


# revision 30
# speedup vs baseline: 1.0090x; 1.0090x over previous
"""Trainium2 Bass kernel for the GNN message-passing model.

Strategy: data-parallel over batch (B=16 -> 2 batches per core, 8 cores, no
cross-core communication). The adjacency matrix is pre-scaled by N (its
entries are ~1/N, below the fp8 subnormal range), cast to fp8-e4m3 on the
host and kept FULLY RESIDENT in SBUF (16 MB -> 128 KB/partition). The
adjacency matmul runs in fp8 DoubleRow perf mode (K=256 per matmul) with
the cut features as the stationary operand, scaled per layer into fp8
range; scales are divided back out in the PSUM drain. Cut features are
produced directly node-major (stationary = x node-tile, moving = gw cut
columns), eliminating all PE transposes. Half of each layer's adjacency
matmul (i-block group 0) is interleaved under the producer loop that
builds the stationary features, keeping the PE dense (HAM-warm) and
hiding ~16 us per layer.

Host-side folds: pw3/emb folded into gw0; action-MLP output folded into
the embedding table via the one-hot rows summing to 1; the nerf frequency
scaling/replication of mesh is a host-side linear layout transform.
"""

import numpy as np
import ml_dtypes

import concourse.bass as bass
import concourse.mybir as mybir
import concourse.tile as tile
from concourse.masks import make_identity
from concourse.bass_utils import run_bass_kernel_spmd

F32 = mybir.dt.float32
BF16 = mybir.dt.bfloat16
FP8 = mybir.dt.float8e4
AF = mybir.ActivationFunctionType
ALU = mybir.AluOpType
PM = mybir.MatmulPerfMode
BF = ml_dtypes.bfloat16
F8 = ml_dtypes.float8_e4m3

B, N, BC = 16, 4096, 2          # batches, nodes, batches per core
NCORES = 8
NB = 8                          # i-blocks / chunks of 512 columns
NQ = 32                         # node tiles of 128
NT = 16                         # double-j-tiles of 256 rows
MAGIC = float(1.5 * 2 ** 23)    # fp32 round-to-nearest magic constant
TWO_PI = float(2.0 * np.pi)
SA = 4096.0                     # adj pre-scale (host)
SF = [256.0, 512.0, 1024.0, 1024.0]   # f_cut fp8 scales per layer
CD = [42, 42, 42, 50]           # cut widths

# const blob layouts: name -> (rows, col offset, cols)
O16 = {"pw1p": (67, 0, 25), "pw2": (25, 25, 50), "w3f": (50, 75, 128),
       "t4": (4, 203, 128), "gw1": (128, 331, 128), "gw2": (128, 459, 128),
       "gw3": (128, 587, 50), "ones4": (2, 637, 8)}
W16 = 645
O32 = {"aw1a": (50, 0, 128), "aw1b": (50, 128, 72), "aw2a": (128, 200, 100),
       "aw2b": (72, 300, 100), "aw3": (100, 400, 100),
       "gw0L": (100, 500, 128), "biasd": (128, 628, 12)}
W32 = 640

run_kwargs = {}                 # test.py may inject trace kwargs here


def split_excess_waits(nc, max_waits=1):
    """Walrus codegen on this image rejects >1 sem wait per instruction;
    move excess waits onto preceding same-engine no-ops."""
    n_split = 0
    for fn in nc.m.functions:
        for blk in fn.blocks:
            insts = list(blk.instructions)
            out = []
            changed = False
            for inst in insts:
                si = getattr(inst, "sync_info", None)
                if si is not None and len(si.on_wait) > max_waits:
                    waits = list(si.on_wait)
                    chunks = [waits[i:i + max_waits]
                              for i in range(0, len(waits), max_waits)]
                    for ci, ch in enumerate(chunks[:-1]):
                        nop = mybir.InstNoOp(
                            name=f"{inst.name}-wsplit-{ci}", ins=[], outs=[])
                        nop.engine = inst.engine
                        nop.sync_info = mybir.SyncInfo(on_wait=ch, on_update=[])
                        out.append(nop)
                        n_split += 1
                    inst.sync_info = mybir.SyncInfo(
                        on_wait=chunks[-1], on_update=list(si.on_update))
                    changed = True
                out.append(inst)
            if changed:
                blk.instructions = out
    return n_split


def build_bass(split=True):
    nc = bass.Bass()

    def _param(name, shape, dt):
        return nc.declare_dram_parameter(name, list(shape), dt, isOutput=False)

    adj8d = _param("adj8d", [128, NQ * N], FP8)
    meshTd = _param("meshTd", [BC, 3, N], F32)
    mshftd = _param("mshftd", [128, N], F32)
    onehotd = _param("onehotd", [4, BC * N], BF16)
    maskTd = _param("maskTd", [50, BC], F32)
    wb16d = _param("wb16d", [128, W16], BF16)
    wb32d = _param("wb32d", [128, W32], F32)
    outd = nc.declare_dram_parameter("outd", [114, 1], F32, isOutput=True)

    with tile.TileContext(nc) as tc:
        _emit(nc, tc, locals())
    if split:
        split_excess_waits(nc)
    return nc


def _emit(nc, tc, d):
    import contextlib
    ctx = contextlib.ExitStack()
    meshTd, onehotd, outd, mshftd = (d["meshTd"], d["onehotd"], d["outd"],
                                     d["mshftd"])

    adjp = ctx.enter_context(tc.tile_pool(name="adjp", bufs=1))
    cpool = ctx.enter_context(tc.tile_pool(name="consts", bufs=1))
    actp = ctx.enter_context(tc.tile_pool(name="acts", bufs=1))
    fcp = ctx.enter_context(tc.tile_pool(name="fcp", bufs=2))
    wkp = ctx.enter_context(tc.tile_pool(name="wkp", bufs=2))
    smallp = ctx.enter_context(tc.tile_pool(name="small", bufs=2))

    psf = ctx.enter_context(tc.tile_pool(name="psf", bufs=2, space="PSUM"))
    psc = ctx.enter_context(tc.tile_pool(name="psc", bufs=2, space="PSUM"))
    psl = ctx.enter_context(tc.tile_pool(name="psl", bufs=4, space="PSUM"))

    # ------- small latency-critical DMAs first (sync queue head) -------
    wb16 = cpool.tile([128, W16], BF16, tag="wb16")
    nc.sync.dma_start(out=wb16[:], in_=d["wb16d"][:])
    wb32 = cpool.tile([128, W32], F32, tag="wb32")
    nc.sync.dma_start(out=wb32[:], in_=d["wb32d"][:])
    maskT = cpool.tile([50, BC], F32, tag="maskT")
    nc.sync.dma_start(out=maskT[:], in_=d["maskTd"][:])
    onehot = cpool.tile([4, BC * N], BF16, tag="onehot")
    nc.sync.dma_start(out=onehot[:], in_=onehotd[:])

    # ------- resident fp8 adjacency streams behind them on sync -------
    adjsb = adjp.tile([128, NQ * N], FP8, tag="adj")
    a3 = adjsb[:].rearrange("p (q c) -> p q c", c=N)
    a3d = d["adj8d"][:].rearrange("p (q c) -> p q c", c=N)
    for g in range(8):
        nc.sync.dma_start(out=a3[:, 4 * g:4 * g + 4, :],
                          in_=a3d[:, 4 * g:4 * g + 4, :])

    def c16(key):
        r, o, w = O16[key]
        return wb16[0:r, o:o + w]

    def c32(key):
        r, o, w = O32[key]
        return wb32[0:r, o:o + w]

    ident = cpool.tile([128, 128], BF16, tag="ident")
    make_identity(nc, ident[:])
    pw1, pw2, w3f, t4 = c16("pw1p"), c16("pw2"), c16("w3f"), c16("t4")
    ones4 = c16("ones4")
    gws = [None, c16("gw1"), c16("gw2"), c16("gw3")]
    biases = c32("biasd")

    def bcol(col, p0, p1):
        return biases[p0:p1, col:col + 1]

    # ---------------- activation tiles ----------------
    xt = actp.tile([128, BC * N], BF16, tag="xt")      # [feat, b*N+n]
    mx = actp.tile([114, NB], F32, tag="mx")
    nc.gpsimd.memset(mx[:], 0.0)
    outsb = actp.tile([114, 1], F32, tag="outsb")
    cvec = actp.tile([128, BC], BF16, tag="cvec")
    cvT = actp.tile([2, 128], BF16, tag="cvT")
    t4b = [actp.tile([4, 128], BF16, tag=f"t4b{b}", name=f"t4b{b}")
           for b in range(BC)]

    # ---------------- action MLP (tiny, fp32) ----------------
    pa = psf.tile([128, 2], F32, tag="f")
    nc.tensor.matmul(pa[:], lhsT=c32("aw1a"), rhs=maskT[:], start=True,
                     stop=True)
    a1a = smallp.tile([128, 2], F32, tag="a1a")
    nc.scalar.activation(a1a[:], pa[:], AF.Relu, bias=bcol(0, 0, 128))
    pb = psf.tile([72, 2], F32, tag="f")
    nc.tensor.matmul(pb[:], lhsT=c32("aw1b"), rhs=maskT[:], start=True,
                     stop=True)
    a1b = smallp.tile([72, 2], F32, tag="a1b")
    nc.scalar.activation(a1b[:], pb[:], AF.Relu, bias=bcol(1, 0, 72))
    pc_ = psf.tile([100, 2], F32, tag="f")
    nc.tensor.matmul(pc_[:], lhsT=c32("aw2a"), rhs=a1a[:], start=True,
                     stop=False)
    nc.tensor.matmul(pc_[:], lhsT=c32("aw2b"), rhs=a1b[:], start=False,
                     stop=True)
    a2 = smallp.tile([100, 2], F32, tag="a2")
    nc.scalar.activation(a2[:], pc_[:], AF.Relu, bias=bcol(2, 0, 100))
    pd = psf.tile([100, 2], F32, tag="f")
    nc.tensor.matmul(pd[:], lhsT=c32("aw3"), rhs=a2[:], start=True, stop=True)
    a3t = smallp.tile([100, 2], F32, tag="a3t")
    nc.scalar.activation(a3t[:], pd[:], AF.Identity, bias=bcol(3, 0, 100))
    pe_ = psf.tile([128, 2], F32, tag="f")
    nc.tensor.matmul(pe_[:], lhsT=c32("gw0L"), rhs=a3t[:], start=True,
                     stop=True)
    nc.scalar.activation(cvec[:], pe_[:], AF.Identity, bias=bcol(4, 0, 128))

    # cvec -> cvT (transpose), then t4b[b] = t4 + cvT[b] (one-hot rows sum
    # to 1, so adding cvec to every embedding row injects the per-batch
    # action constant into the layer-0 matmul)
    pt = psc.tile([2, 128], BF16, tag="c")
    nc.tensor.transpose(pt[:], cvec[:], ident[:])
    nc.vector.tensor_copy(cvT[:], pt[:])
    for b in range(BC):
        p4 = psc.tile([4, 128], F32, tag="c")
        nc.tensor.matmul(p4[:], lhsT=ones4[:, 4 * b:4 * b + 4], rhs=cvT[:],
                         start=True, stop=False)
        nc.tensor.matmul(p4[:], lhsT=ident[0:4, 0:4], rhs=t4, start=False,
                         stop=True)
        nc.vector.tensor_copy(t4b[b][:], p4[:])

    # fcst: [128 nodes-in-tile, NT blocks of (2 half, 128 m)] fp8 stationary
    fcst, fvs = [None] * 4, [None] * 4

    def alloc_fcst(li):
        fcst[li] = fcp.tile([128, NT * 256], FP8, tag="fc", name=f"fcst{li}")
        fvs[li] = fcst[li][:].rearrange("p (t h m) -> p t h m", h=2, m=128)
        # zero the gap columns between the two batches' packed features
        nc.vector.memset(fvs[li][:, :, :, 42:64], 0.0)

    def phase_c_mms(li, ts, pls, ib0):
        mm = 64 + CD[li]
        for t in ts:
            lhsT = fvs[li][:, t, :, 0:mm]
            for k in range(len(pls)):
                ib = ib0 + k
                nc.tensor.matmul(
                    pls[k][:], lhsT=lhsT,
                    rhs=a3[:, 2 * t:2 * t + 2, ib * 512:(ib + 1) * 512],
                    start=(t == 0), stop=(t == NT - 1),
                    perf_mode=PM.DoubleRow)

    def drains(li, pls, ib0):
        s_li = 1.0 / (SA * SF[li])
        for k in range(len(pls)):
            ib = ib0 + k
            if li < 3:
                for b in range(BC):
                    ics = slice(b * N + ib * 512, b * N + (ib + 1) * 512)
                    nc.scalar.activation(
                        xt[0:42, ics], pls[k][64 * b:64 * b + 42, :],
                        AF.Relu, bias=bcol(7 + li, 64 * b, 64 * b + 42),
                        scale=s_li)
            else:
                nc.vector.tensor_reduce(mx[0:114, ib:ib + 1], pls[k][:],
                                        mybir.AxisListType.X, ALU.max)

    def new_pls(li, g, n=4):
        mm = 64 + CD[li]
        return [psl.tile([mm, 512], F32, tag="L", name=f"pl{li}_{g}_{k}")
                for k in range(n)]

    # ---------------- front-end + layer-0 phase A/cut (fused),
    # with layer-0 phase C group 0 interleaved ----------------
    alloc_fcst(0)
    pls0 = new_pls(0, 0)

    def stage0(ch):
        # t2[64b+k] = mesh[b,k%3]*freq[k//3]/(2pi) (+0.25 on cos rows),
        # prebuilt host-side; gpsimd DMA so the adj stream on the sync
        # queue cannot delay it. Emitted one chunk ahead so the DVE/ACT
        # chain for chunk ch+1 overlaps chunk ch's matmuls.
        cs = slice(ch * 512, (ch + 1) * 512)
        t2 = wkp.tile([128, 512], F32, tag="t2")
        nc.gpsimd.dma_start(out=t2[:], in_=mshftd[:, cs])
        # range reduce: rB = round(t2) - t2 = -frac; sin(-2pi*rB)=sin(2pi*t2)
        rA = wkp.tile([128, 512], F32, tag="rA")
        nc.vector.tensor_scalar_add(rA[:], t2[:], MAGIC)
        rB = wkp.tile([128, 512], F32, tag="rB")
        nc.vector.scalar_tensor_tensor(rB[:], rA[:], MAGIC, t2[:],
                                       op0=ALU.subtract, op1=ALU.subtract)
        peins = []
        for b in range(BC):
            pein = wkp.tile([67, 512], BF16, tag=f"pein{b}", name=f"pein{b}")
            nc.gpsimd.dma_start(out=pein[64:67, :], in_=meshTd[b, :, cs])
            peins.append(pein)
        return rB, peins

    nxt = stage0(0)
    for ch in range(NB):
        cs = slice(ch * 512, (ch + 1) * 512)
        rB, peins = nxt
        h1s, h2s, ph1s, ph2s, pcus, pfs = [], [], [], [], [], []
        for b in range(BC):
            nc.scalar.activation(peins[b][0:64, :], rB[64 * b:64 * b + 64, :],
                                 AF.Sin, scale=-TWO_PI)
        if ch + 1 < NB:
            nxt = stage0(ch + 1)
        for b in range(BC):
            ph1 = psf.tile([25, 512], F32, tag="f")
            nc.tensor.matmul(ph1[:], lhsT=pw1, rhs=peins[b][:], start=True,
                             stop=True)
            ph1s.append(ph1)
        for b in range(BC):
            h1 = wkp.tile([25, 512], BF16, tag=f"h1{b}", name=f"h1{b}")
            nc.vector.tensor_scalar(h1[:], ph1s[b][:], bcol(5, 0, 25), 0.0,
                                    op0=ALU.add, op1=ALU.max)
            h1s.append(h1)
        for b in range(BC):
            ph2 = psf.tile([50, 512], F32, tag="f")
            nc.tensor.matmul(ph2[:], lhsT=pw2, rhs=h1s[b][:], start=True,
                             stop=True)
            ph2s.append(ph2)
        for b in range(BC):
            h2 = wkp.tile([50, 512], BF16, tag=f"h2{b}", name=f"h2{b}")
            nc.scalar.activation(h2[:], ph2s[b][:], AF.Relu,
                                 bias=bcol(6, 0, 50))
            h2s.append(h2)
        # layer-0 cut, node-major: fcT[n,c] = h2.T @ w3f_cut + oh.T @ t4b_cut
        for b in range(BC):
            pcu = psc.tile([128, 4 * 42], F32, tag="c")
            for j in range(4):
                q = 4 * ch + j
                js = slice(42 * j, 42 * j + 42)
                nc.tensor.matmul(pcu[:, js],
                                 lhsT=h2s[b][:, 128 * j:128 * j + 128],
                                 rhs=w3f[:, 0:42], start=True, stop=False)
                nc.tensor.matmul(
                    pcu[:, js],
                    lhsT=onehot[:, b * N + q * 128:b * N + (q + 1) * 128],
                    rhs=t4b[b][:, 0:42], start=False, stop=True)
            pcus.append(pcu)
        for b in range(BC):
            nc.vector.tensor_scalar_mul(
                fvs[0][:, 2 * ch:2 * ch + 2, :, 64 * b:64 * b + 42],
                pcus[b][:].rearrange("p (t h c) -> p t h c", h=2, c=42),
                SF[0])
        # layer-0 right features
        for b in range(BC):
            xs = slice(b * N + ch * 512, b * N + (ch + 1) * 512)
            pf = psf.tile([128, 512], F32, tag="f")
            nc.tensor.matmul(pf[:], lhsT=w3f, rhs=h2s[b][:], start=True,
                             stop=False)
            nc.tensor.matmul(pf[:], lhsT=t4b[b][:], rhs=onehot[:, xs],
                             start=False, stop=True)
            pfs.append(pf)
        for b in range(BC):
            xs = slice(b * N + ch * 512, b * N + (ch + 1) * 512)
            nc.vector.tensor_scalar_max(xt[32:64, xs], pfs[b][32:64, :], 0.0)
            nc.scalar.activation(xt[64:128, xs], pfs[b][64:128, :], AF.Relu)
        phase_c_mms(0, (2 * ch, 2 * ch + 1), pls0, ib0=0)

    # ---------------- GCN layers ----------------
    for li in range(4):
        if li > 0:
            # phase A (cut + right) with phase C group 0 interleaved
            cd = CD[li]
            alloc_fcst(li)
            pls0 = new_pls(li, 0)
            for ch in range(NB):
                pcus, pfs = [], []
                for b in range(BC):
                    pcu = psc.tile([128, 4 * cd], F32, tag="c")
                    for j in range(4):
                        q = 4 * ch + j
                        nc.tensor.matmul(
                            pcu[:, cd * j:cd * j + cd],
                            lhsT=xt[:, b * N + q * 128:b * N + (q + 1) * 128],
                            rhs=gws[li][:, 0:cd], start=True, stop=True)
                    pcus.append(pcu)
                if li < 3:
                    for b in range(BC):
                        xs = slice(b * N + ch * 512, b * N + (ch + 1) * 512)
                        pf = psf.tile([128, 512], F32, tag="f")
                        nc.tensor.matmul(pf[:], lhsT=gws[li], rhs=xt[:, xs],
                                         start=True, stop=True)
                        pfs.append(pf)
                for b in range(BC):
                    nc.vector.tensor_scalar_mul(
                        fvs[li][:, 2 * ch:2 * ch + 2, :, 64 * b:64 * b + cd],
                        pcus[b][:].rearrange("p (t h c) -> p t h c",
                                             h=2, c=cd), SF[li])
                if li < 3:
                    for b in range(BC):
                        xs = slice(b * N + ch * 512, b * N + (ch + 1) * 512)
                        nc.vector.tensor_scalar_max(xt[32:64, xs],
                                                    pfs[b][32:64, :], 0.0)
                        nc.scalar.activation(xt[64:128, xs],
                                             pfs[b][64:128, :], AF.Relu)
                phase_c_mms(li, (2 * ch, 2 * ch + 1), pls0, ib0=0)
        # phase C group 0 drains, then remaining i-blocks dense. The last
        # layer's tail is split so its final drains overlap matmuls.
        drains(li, pls0, ib0=0)
        if li < 3:
            pls1 = new_pls(li, 1)
            phase_c_mms(li, range(NT), pls1, ib0=4)
            drains(li, pls1, ib0=4)
        else:
            pls1 = new_pls(li, 1, n=2)
            phase_c_mms(li, range(NT), pls1, ib0=4)
            pls2 = new_pls(li, 2, n=2)
            phase_c_mms(li, range(NT), pls2, ib0=6)
            drains(li, pls1, ib0=4)
            drains(li, pls2, ib0=6)

    # ---------------- final max + scale + bias + output ----------------
    mxr = smallp.tile([114, 1], F32, tag="mxr")
    nc.vector.tensor_reduce(mxr[:], mx[:], mybir.AxisListType.X, ALU.max)
    nc.scalar.activation(outsb[:], mxr[:], AF.Identity,
                         bias=bcol(10, 0, 114), scale=1.0 / (SA * SF[3]))
    nc.sync.dma_start(out=outd[:], in_=outsb[:])
    ctx.close()


# ---------------------------------------------------------------------------
# host side
# ---------------------------------------------------------------------------

def _prep_shared(inp):
    """Host preprocessing shared across cores (weights + adj)."""
    f32 = np.float32
    adjT = np.ascontiguousarray(inp["adj"].astype(f32).T)
    adj8 = np.clip(adjT * SA, 0, 240).astype(F8)            # [j, i]
    adj8 = np.ascontiguousarray(
        adj8.reshape(NQ, 128, N).transpose(1, 0, 2)).reshape(128, NQ * N)

    gw0 = inp["gw0"].astype(f32)
    w3fold = inp["pw3"].astype(f32) @ gw0[100:200]
    t4 = inp["emb"].astype(f32) @ gw0[200:300]
    pb3f = inp["pb3"].astype(f32) @ gw0[100:200]

    # pe_in row permutation: ours = [sin(f,c) x30 | cos(f,c) x30 | mesh x3]
    pw1f = inp["pw1"].astype(f32)
    pw1p = np.zeros((67, 25), f32)
    for k in range(30):
        f, c = divmod(k, 3)
        pw1p[k] = pw1f[f * 6 + c]          # sin rows
        pw1p[32 + k] = pw1f[f * 6 + 3 + c]  # cos rows
    pw1p[64:67] = pw1f[60:63]

    biasd = np.zeros((128, 12), f32)
    biasd[0:128, 0] = inp["ab1"][:128]
    biasd[0:72, 1] = inp["ab1"][128:200]
    biasd[0:100, 2] = inp["ab2"]
    biasd[0:100, 3] = inp["ab3"]
    biasd[0:128, 4] = pb3f
    biasd[0:25, 5] = inp["pb1"].astype(f32)
    biasd[0:50, 6] = inp["pb2"].astype(f32)
    for li in range(3):
        biasd[0:42, 7 + li] = inp[f"gb{li}"].astype(f32)[:42]
        biasd[64:106, 7 + li] = inp[f"gb{li}"].astype(f32)[:42]
    biasd[0:50, 10] = inp["gb3"].astype(f32)
    biasd[64:114, 10] = inp["gb3"].astype(f32)

    ones4 = np.zeros((2, 8), f32)
    ones4[0, 0:4] = 1.0
    ones4[1, 4:8] = 1.0

    vals16 = {"pw1p": pw1p, "pw2": inp["pw2"].astype(f32), "w3f": w3fold,
              "t4": t4, "gw1": inp["gw1"].astype(f32),
              "gw2": inp["gw2"].astype(f32), "gw3": inp["gw3"].astype(f32),
              "ones4": ones4}
    wb16 = np.zeros((128, W16), f32)
    for k, (r, o, w) in O16.items():
        wb16[0:r, o:o + w] = vals16[k]
    vals32 = {"aw1a": inp["aw1"].astype(f32)[:, :128],
              "aw1b": inp["aw1"].astype(f32)[:, 128:200],
              "aw2a": inp["aw2"].astype(f32)[:128],
              "aw2b": inp["aw2"].astype(f32)[128:200],
              "aw3": inp["aw3"].astype(f32),
              "gw0L": gw0[:100], "biasd": biasd}
    wb32 = np.zeros((128, W32), f32)
    for k, (r, o, w) in O32.items():
        wb32[0:r, o:o + w] = vals32[k]

    return {
        "adj8d": adj8,
        "wb16d": wb16.astype(BF),
        "wb32d": wb32,
    }


def _prep_core(inp, shared, core):
    bs = slice(core * BC, (core + 1) * BC)
    f32 = np.float32
    mesh = inp["mesh"].astype(f32)[bs]                       # [2, N, 3]
    meshT = np.ascontiguousarray(mesh.transpose(0, 2, 1))    # [2, 3, N]
    mi = inp["mask_idx"][bs]                                 # [2, N] int32
    onehot = (mi[:, None, :] == np.arange(4, dtype=mi.dtype)[None, :, None])
    onehot = np.ascontiguousarray(
        onehot.transpose(1, 0, 2).reshape(4, BC * N)).astype(BF)
    maskT = np.ascontiguousarray(inp["mask"].astype(f32)[bs].T)  # [50, 2]
    # t2 rows, prebuilt: rows 64b+k = freq2[k]*mesh[b, k%3]; +0.25 on cos rows
    freqs = np.asarray([np.pi] + [2.0 * np.pi * i for i in range(1, 10)], f32)
    freq2 = np.repeat(freqs, 3) / (2.0 * np.pi)              # [30]
    mshft = np.zeros((128, N), f32)
    for b in range(BC):
        rep = freq2[:, None] * meshT[b][np.tile(np.arange(3), 10)]  # [30, N]
        mshft[64 * b:64 * b + 30] = rep
        mshft[64 * b + 32:64 * b + 62] = rep + 0.25
    m = dict(shared)
    m["meshTd"] = meshT
    m["mshftd"] = mshft
    m["onehotd"] = onehot
    m["maskTd"] = maskT
    return m


_CACHED = {}


def kernel(**inputs) -> np.ndarray:
    if "nc" not in _CACHED:
        _CACHED["nc"] = build_bass()
    nc = _CACHED["nc"]
    shared = _prep_shared(inputs)
    in_maps = [_prep_core(inputs, shared, c) for c in range(NCORES)]
    res = run_bass_kernel_spmd(nc, in_maps, list(range(NCORES)), **run_kwargs)
    out = np.empty((B, 50), np.float32)
    for c in range(NCORES):
        o = res.results[c]["outd"][:, 0]
        out[2 * c] = o[0:50]
        out[2 * c + 1] = o[64:114]
    _CACHED["last_results"] = res
    return out
